# revision 1
# baseline (speedup 1.0000x reference)
"""4D multilinear interpolation (8^4 lattice) on 8 Trainium2 cores — v3.

Fully pipelined raw-bass kernel:
  - 32 indirect DMA gathers (InstDMACopy, resident DGE ucode — no ext-isa
    library load), each fetching per partition the exact 586-float corner
    span of one row.  Gathers are gen-bound (~1.2us each on the Q7 SWDGE);
    they start as soon as the DVE finishes the index computation (~10us)
    and run back-to-back (one dedicated completion sem per gather, no
    artificial serialization).
  - Index math on DVE: Horner form idx = ((f0*8+f1)*8+f2)*8+f3 plus a
    host-provided flat row-base table (row*4096, exact in f32 since
    4096*4096 = 2^24).  No iota — avoids any gpsimd library load.
  - Corner extraction at offset 0 (span starts exactly at the cell corner),
    so the blend per gather is one [128,16] strided mult against W16 and
    one XYZW tensor_reduce to [128,1] — minimal DVE occupancy, software-
    pipelined 4 deep so semaphore waits land on long-retired producers.
  - Per-8-gather output stores on the Sync (HWDGE) engine.

Slot (p, g) holds row 128*g + p of the core's slice; mesh_pred needs no
host permutation; coordinates are host-permuted into (p, g) order.
"""

from contextlib import ExitStack

import numpy as np

import concourse.bass as bass
import concourse.bacc as bacc
import concourse.mybir as mybir
from concourse import bass_utils

F32 = mybir.dt.float32
I32 = mybir.dt.int32
OP = mybir.AluOpType

P = 128
NG = 32           # gathers (row groups) per core
ND = 4
VOL = 4096
NCORES = 8
BC = P * NG
SPAN = 586        # corner span in f32 (585 max offset + 1)
SEGW = 592        # padded per-gather width (32B aligned)
PIPE = 4          # blend software-pipeline depth


def _v(t, off, dims):
    ap = t[:]
    return bass.AP(ap.tensor, ap.offset + off, [ap.ap[0], *dims])


def _build():
    nc = bacc.Bacc("TRN2", target_bir_lowering=False, debug=False)
    mesh = nc.dram_tensor("mesh_pred", [BC, VOL], F32, kind="ExternalInput")
    # wc: [coords (p,g,d) | T2 row-base table] -> [128, 160] f32
    wc_d = nc.dram_tensor("wc", [P, NG * ND + NG], F32, kind="ExternalInput")
    out_d = nc.dram_tensor("out", [P, NG], F32, kind="ExternalOutput")

    with (
        nc.Block() as block,
        ExitStack() as stack,
    ):
        sb = lambda name, shape, dt=F32: stack.enter_context(
            nc.sbuf_tensor(name, shape, dt)
        )
        WC = sb("WC", [P, NG * ND + NG])
        C4 = sb("C4", [P, NG * ND])
        GE = sb("GE", [P, 6 * NG * ND])
        FL = sb("FL", [P, NG * ND])
        XH = sb("XH", [P, NG])
        FLI = sb("FLI", [P, NG * ND], I32)
        IDXI = sb("IDXI", [P, NG], I32)
        OM = sb("OM", [P, NG * ND])
        W01 = sb("W01", [P, 4 * NG])
        W23 = sb("W23", [P, 4 * NG])
        W16 = sb("W16", [P, NG * 16])
        G = sb("G", [P, NG * SEGW])
        M16 = sb("M16", [P, NG * 16])
        ACC = sb("ACC", [P, NG])
        lsem = stack.enter_context(nc.semaphore("lsem"))
        isem = stack.enter_context(nc.semaphore("isem"))
        dsem = stack.enter_context(nc.semaphore("dsem"))
        osem = stack.enter_context(nc.semaphore("osem"))
        vsem = stack.enter_context(nc.semaphore("vsem"))
        gsem = [stack.enter_context(nc.semaphore(f"g{g}")) for g in range(NG)]

        @block.sync
        def _(sync: bass.BassEngine):
            sync.dma_start(WC[:], wc_d[:]).then_inc(lsem, 16)
            for k in range(4):
                sync.wait_ge(dsem, 8 * (k + 1))
                sync.dma_start(
                    out_d[:, 8 * k : 8 * (k + 1)], ACC[:, 8 * k : 8 * (k + 1)]
                ).then_inc(osem, 16)
            sync.wait_ge(osem, 64)

        @block.gpsimd
        def _(gp: bass.BassGpSimd):
            mesh_2d = mesh[:]
            for g in range(NG):
                if g in (0, 8):
                    gp.wait_ge(isem, 1 if g == 0 else 2)
                gp.indirect_dma_start(
                    out=_v(G, g * SEGW, [[1, SPAN]]),
                    out_offset=None,
                    in_=mesh_2d,
                    in_offset=bass.IndirectOffsetOnAxis(
                        ap=IDXI[:, g : g + 1], axis=1
                    ),
                    element_offset=0,
                ).then_inc(gsem[g], 16)

        @block.vector
        def _(ve: bass.BassEngine):
            state = {"n": 0}

            def op(fn, *a, **kw):
                inst = fn(*a, **kw).then_inc(vsem, 1)
                state["n"] += 1
                return inst

            def bar():
                ve.wait_ge(vsem, state["n"])

            ve.wait_ge(lsem, 16)  # WC in

            # --- index pipeline, split so early gathers start sooner ---
            # floor(7x) = int32(7x - 0.5): HW f32->i32 converts round-to-
            # nearest (measured), and round(c - 0.5) = floor(c); exact-integer
            # c ties resolve to ci = c-1, frac = 1.0 which interpolates to the
            # identical value by continuity.  c < 7 strictly, so ci <= 6.
            def idx_chain(g0, gw):
                c0, cw = 4 * g0, 4 * gw
                cs = lambda t: _v(t, c0, [[1, cw]])
                op(ve.tensor_scalar, out=cs(C4), in0=_v(WC, c0, [[1, cw]]),
                   scalar1=7.0, scalar2=-0.5, op0=OP.mult, op1=OP.add)
                bar()
                op(ve.tensor_copy, out=_v(FLI, c0, [[1, cw]]), in_=cs(C4))
                bar()
                # Horner in int32: idx = ((F0*8+F1)*8+F2)*8+F3 + row-base
                # (row-base table arrives as int32 bits in the f32 wc tensor)
                fdi = lambda d: _v(FLI, c0 + d, [[4, gw]])
                xhi = _v(IDXI, g0, [[1, gw]])
                op(ve.scalar_tensor_tensor, xhi, fdi(0), 8, fdi(1),
                   op0=OP.mult, op1=OP.add)
                bar()
                op(ve.scalar_tensor_tensor, xhi, xhi, 8, fdi(2),
                   op0=OP.mult, op1=OP.add)
                bar()
                op(ve.scalar_tensor_tensor, xhi, xhi, 8, fdi(3),
                   op0=OP.mult, op1=OP.add)
                bar()
                wci = WC[:].bitcast(I32)
                op(ve.tensor_tensor, out=xhi, in0=xhi,
                   in1=bass.AP(wci.tensor, wci.offset + NG * ND + g0,
                               [wci.ap[0], [1, gw]]), op=OP.add)
                bar()
                ve.sem_inc(isem, 1)

            idx_chain(0, 8)
            idx_chain(8, NG - 8)

            # --- weights: W16[(g, 8i+4j+2k+l)] = wx_i wy_j wz_k ww_l ---
            op(ve.tensor_copy, out=FL[:], in_=FLI[:])
            bar()
            # frac = (C4 + 0.5) - FL   (C4 holds 7x - 0.5)
            op(ve.scalar_tensor_tensor, FL[:], FL[:], -1.0, C4[:],
               op0=OP.mult, op1=OP.add)
            bar()
            op(ve.tensor_scalar, out=FL[:], in0=FL[:], scalar1=0.5, scalar2=None,
               op0=OP.add)
            bar()
            op(ve.tensor_scalar, out=OM[:], in0=FL[:], scalar1=-1.0, scalar2=1.0,
               op0=OP.mult, op1=OP.add)
            bar()
            pairs = ((0, 0), (0, 1), (1, 0), (1, 1))
            for q, (a, b) in enumerate(pairs):
                op(ve.tensor_tensor,
                   out=W01[:, q * NG : (q + 1) * NG],
                   in0=_v(FL if a else OM, 0, [[4, NG]]),
                   in1=_v(FL if b else OM, 1, [[4, NG]]), op=OP.mult)
                op(ve.tensor_tensor,
                   out=W23[:, q * NG : (q + 1) * NG],
                   in0=_v(FL if a else OM, 2, [[4, NG]]),
                   in1=_v(FL if b else OM, 3, [[4, NG]]), op=OP.mult)
            bar()
            for kc in range(16):
                q, r = kc >> 2, kc & 3
                op(ve.tensor_tensor,
                   out=_v(W16, kc, [[16, NG]]),
                   in0=W01[:, q * NG : (q + 1) * NG],
                   in1=W23[:, r * NG : (r + 1) * NG], op=OP.mult)
            bar()

            # --- software-pipelined per-gather blend ---
            vc = {}
            for t in range(NG + PIPE):
                if t >= PIPE:
                    g = t - PIPE
                    ve.wait_ge(vsem, vc[g])
                    ve.tensor_reduce(
                        out=_v(ACC, g, [[1, 1]]),
                        in_=_v(M16, 16 * g, [[1, 16]]),
                        axis=mybir.AxisListType.X, op=OP.add,
                    ).then_inc(dsem, 1)
                if t < NG:
                    ve.wait_ge(gsem[t], 16)
                    for i in range(2):  # dim-0 planes (3D ISA pattern limit)
                        op(ve.tensor_tensor,
                           out=_v(M16, 16 * t + 8 * i, [[4, 2], [2, 2], [1, 2]]),
                           in0=_v(G, t * SEGW + 512 * i,
                                  [[64, 2], [8, 2], [1, 2]]),
                           in1=_v(W16, 16 * t + 8 * i, [[4, 2], [2, 2], [1, 2]]),
                           op=OP.mult)
                    vc[t] = state["n"]

    nc.compile()
    return nc


_NC = None


def _get_nc():
    global _NC
    if _NC is None:
        _NC = _build()
    return _NC


def _host_tables(cs):
    """cs: [4096, 4] f32 -> wc [128, 160] f32 (coords (p,g,d) + row base)."""
    cm = cs.reshape(NG, P, ND).transpose(1, 0, 2).reshape(P, NG * ND)
    t2 = (
        (np.arange(P, dtype=np.int64)[:, None]
         + 128 * np.arange(NG, dtype=np.int64)[None, :]) * VOL
    ).astype(np.int32).view(np.float32)
    return np.ascontiguousarray(
        np.concatenate([cm, t2], axis=1).astype(np.float32)
    )


def kernel(coordinates, mesh_pred, _trace=False, _tmpdir=None):
    coordinates = np.asarray(coordinates, dtype=np.float32)
    mesh_pred = np.asarray(mesh_pred, dtype=np.float32)
    assert coordinates.shape == (NCORES * BC, ND)
    assert mesh_pred.shape == (NCORES * BC, VOL)

    in_maps = []
    for cix in range(NCORES):
        sl = slice(cix * BC, (cix + 1) * BC)
        in_maps.append(
            {
                "mesh_pred": np.ascontiguousarray(mesh_pred[sl]),
                "wc": _host_tables(coordinates[sl]),
            }
        )
    res = bass_utils.run_bass_kernel_spmd(
        _get_nc(), in_maps, core_ids=list(range(NCORES)), trace=_trace,
        tmpdir=_tmpdir,
    )
    outs = []
    for r in res.results:
        o = np.asarray(r["out"]).reshape(P, NG)  # [p, g]
        outs.append(o.transpose(1, 0).reshape(-1))  # b = g*128 + p
    out = np.concatenate(outs)
    if _trace:
        return out, res
    return out

